# revision 26
# baseline (speedup 1.0000x reference)
"""AdaptiveRankingLoss distributed Bass kernel for 8 TRN2 NeuronCores (v2).

Math
----
reference loss = sum_{i<j, t_i != t_j} w_ij * relu(m_ij - sgn(t_i-t_j)(p_i-p_j))
                 / count,   m = 0.1*clip(|t_i-t_j|, 0.1, 1.0), w = 1/(1+u_i+u_j).

Sorting by t and splitting by the sign of as = 0.1*(t_j - t_i) gives the
full-matrix form  sum_{as>0} w * relu(clip(as, .01, .1) - (p_j - p_i)).

Columns are streamed as two fp16 broadcast tensors q = 0.1 t - p and
pn = -p; rows enter as per-partition scalars.  Each 128-row tile (strided
over 1024 consecutive sorted positions, identical span on every core so
one SPMD program serves all 8 cores) splits its column range into zones
classified host-side from the device-exact a-values t01f = q - pn:

  diag  [ind uncertain]          custom DVE op (indicator, max-margin)
  hard  [margin clip uncertain]  custom DVE op (full clip, no indicator)
  L     [m = as exactly]         v = relu(q_j - q_i)        -> stock
                                 tensor_scalar (sub, max0) at 4x DVE mode
  H     [m = 0.1]                v = relu(pn_j + p_i + 0.1) -> ScalarE
                                 activation Relu with per-partition bias

The weight w is applied through the degree-6 bilinear split
w ~ sum_n Phi_n(x_i) Psi_n(x_j), x = u - .5:  TensorEngine contracts
Phi against the v tiles into PSUM X[n, j] (PSUM halves, start/stop
bookkeeping per 512-column bank window), X is drained via a partition
reshape and a fused multiply-reduce against Psi.  Phi and Psi are
precomputed on the host and DMAed in directly.
"""

import numpy as np

import concourse.bass as bass
import concourse.bacc as bacc
import concourse.mybir as mybir
import concourse.tile as tile
from concourse.bass_utils import run_bass_kernel_spmd
from concourse import dve_ops
from concourse.dve_spec import (
    Spec,
    Src0,
    Src1,
    C0,
    C1,
    C2,
    Zero,
    relu,
    maxx,
    minn,
    lower,
    _has_src1,
)
from concourse.dve_uop import DveOpSpec

F32 = mybir.dt.float32
F16 = mybir.dt.float16
BF16 = mybir.dt.bfloat16
ALU = mybir.AluOpType
RELU = mybir.ActivationFunctionType.Relu

N = 8192
NCORES = 8
P = 128
R = N // NCORES      # rows per core (1024)
RT = R // P          # row tiles per core (8)
HW = N // 2          # column half width (4096)
DEG = 6
K = DEG + 1
MMF = 512            # matmul free-dim / PSUM bank window
EPS = 1e-6


# --------------------------------------------------------------------------
# custom DVE ops (Src0 = q col, Src1 = pn col, C0 = t01 row, C1 = p row)
#   a  = (Src0 - Src1) - C0          exact scaled target difference
#   b  = p_j - p_i = -(Src1 + C1)    so  m - b = m + Src1 + C1
# --------------------------------------------------------------------------
def _register_op(name, spec):
    for op in dve_ops.OPS:
        if op.name == name:
            return op
    row = dve_ops._CUSTOM_DVE_ROW_BASE + len(dve_ops.OPS)
    assert row < 0x20, "custom-DVE row overflow"
    dve_ops._SUB_OPCODE_FOR_NAME[name] = row
    shas = {}
    for ver in ("v3", "v4"):
        try:
            uops = lower(spec, ver=ver)
            shas[ver] = DveOpSpec(
                name=name, opcode=row, uops=uops, rd1_en=_has_src1(spec)
            ).sha(ver)
        except Exception:
            pass
    op = dve_ops.DveOp(name, spec, subdim=False, uops_sha=shas)
    dve_ops.OPS.append(op)
    dve_ops.CUSTOM_DVE_SPECS[name] = spec
    return op


def _hard_ref(in0, in1, s0, s1, imm2):
    a = (in0 - in1) - s0
    m = np.clip(a, np.float32(imm2) * np.float32(imm2), imm2)
    return np.maximum(m + in1 + s1, 0.0)


def _diag_ref(in0, in1, s0, s1, imm2):
    a = (in0 - in1) - s0
    m = np.maximum(a, np.float32(imm2) * np.float32(imm2))
    return (a > 0).astype(np.float32) * np.maximum(m + in1 + s1, 0.0)


def _diag2_ref(in0, in1, s0, s1, imm2):
    a = (in0 - in1) - s0
    m = np.minimum(a, np.float32(imm2))
    return (a > 0).astype(np.float32) * np.maximum(m + in1 + s1, 0.0)


_A = (Src0 - Src1) - C0
HARD7 = _register_op(
    "ARL_HARD_V2",
    Spec(body=relu((minn(maxx(_A, C2 * C2), C2) + Src1) + C1), reference=_hard_ref),
)
DIAG8 = _register_op(
    "ARL_DIAG_V2",
    Spec(
        body=(_A > Zero) * relu((maxx(_A, C2 * C2) + Src1) + C1), reference=_diag_ref
    ),
)
DIAG2 = _register_op(
    "ARL_DIAG2_V2",
    Spec(body=(_A > Zero) * relu((minn(_A, C2) + Src1) + C1), reference=_diag2_ref),
)


# --------------------------------------------------------------------------
# degree-6 bilinear split of w = 1/(2 + x_i + x_j), x = u - .5
# --------------------------------------------------------------------------
def _acoef_matrix() -> np.ndarray:
    from numpy.polynomial import chebyshev as _C
    from math import comb

    nodes = np.cos((2 * np.arange(DEG + 1) + 1) / (2 * (DEG + 1)) * np.pi)
    ch = _C.chebfit(nodes, 1.0 / (2.0 + nodes), DEG)
    c = _C.cheb2poly(ch)
    A = np.zeros((K, K), np.float64)
    for mm in range(K):
        for nn in range(K):
            if mm + nn <= DEG:
                A[mm, nn] = c[mm + nn] * comb(mm + nn, mm)
    return A.astype(np.float32)


_ACOEF = _acoef_matrix()


# --------------------------------------------------------------------------
# host-side zone classification (canonical across cores)
# --------------------------------------------------------------------------
def _first_above(arr, val):
    return int(np.searchsorted(arr, val, side="right"))


def _make_schedule(t01f):
    """Per row tile: ordered [(kind, lo, hi)] zones covering [act, N)."""
    cmax = np.maximum.accumulate(t01f)
    csuf = np.minimum.accumulate(t01f[::-1])[::-1]
    tiles = []
    for r in range(RT):
        span = t01f[1024 * r : 1024 * (r + 1)]
        tmin, tmax = float(span.min()), float(span.max())
        act = 0 if r == 0 else _first_above(cmax, tmin - EPS)
        d1 = _first_above(csuf, tmax + EPS)        # a > 0 certain from here
        ds = _first_above(cmax, tmin + 0.1 - EPS)  # diag max-form valid before
        l0 = _first_above(csuf, tmax + 0.01 + EPS)
        l1 = ds
        h0 = _first_above(csuf, tmax + 0.1 + EPS)
        act &= ~1
        d1 = min((d1 + 1) & ~1, N)
        ds_e = min(max((ds + 1) & ~1, act), d1)
        l0 = min(max((l0 + 1) & ~1, d1), N)
        l1 = max(l1 & ~1, l0)
        h0 = min(max((h0 + 1) & ~1, l1), N)
        zones = []
        if ds_e > act:
            zones.append(("diag", act, ds_e))
        if d1 > ds_e:
            zones.append(("diag2", ds_e, d1))
        if l0 > d1:
            zones.append(("hard", d1, l0))
        if l1 > l0:
            zones.append(("L", l0, l1))
        if h0 > l1:
            zones.append(("hard", l1, h0))
        if N > h0:
            zones.append(("H", h0, N))
        tiles.append({"act": act, "zones": zones})
    return tiles


# --------------------------------------------------------------------------
# device graph
# --------------------------------------------------------------------------
def _build_nc(tiles_sched):
    from contextlib import ExitStack

    nc = bacc.Bacc(None, target_bir_lowering=False, debug=False)

    q_ext = nc.declare_dram_parameter("qcol", [N], F16, isOutput=False)
    pn_ext = nc.declare_dram_parameter("pncol", [N], F16, isOutput=False)
    rows_ext = nc.declare_dram_parameter("rows", [P, 5 * RT], F32, isOutput=False)
    phib_ext = nc.declare_dram_parameter("phib", [P, RT * K], BF16, isOutput=False)
    psi_ext = nc.declare_dram_parameter("psiR", [K * 16, 256], BF16, isOutput=False)
    psi56_ext = nc.declare_dram_parameter("psi56", [K * 8, 256], BF16, isOutput=False)
    psip_ext = nc.declare_dram_parameter("psiP", [K, 2048], BF16, isOutput=False)
    out_ext = nc.declare_dram_parameter("out", [448], F32, isOutput=True)

    with tile.TileContext(nc) as tc, ExitStack() as ctx:
        constp = ctx.enter_context(tc.tile_pool(name="const", bufs=1))
        colp = ctx.enter_context(tc.tile_pool(name="cols", bufs=1))
        vp = ctx.enter_context(tc.tile_pool(name="v", bufs=6))
        pp = ctx.enter_context(tc.tile_pool(name="psum", bufs=1, space="PSUM"))
        sp = ctx.enter_context(tc.tile_pool(name="small", bufs=1))

        # ---- column tensors (fp16, broadcast to all partitions) ----
        q_sb = colp.tile([P, N], F16)
        pn_sb = colp.tile([P, N], F16)

        def load_q(lo, w):
            nc.sync.dma_start(
                q_sb[:, lo : lo + w],
                bass.AP(tensor=q_ext, offset=lo, ap=[[0, P], [1, w]]),
            )

        def load_pn(lo, w):
            nc.sync.dma_start(
                pn_sb[:, lo : lo + w],
                bass.AP(tensor=pn_ext, offset=lo, ap=[[0, P], [1, w]]),
            )

        # first wave via the ScalarE HWDGE queue — it is idle right after the
        # entry barrier, ~3 µs before the Sync queue issues its first DMA
        nc.scalar.dma_start(
            pn_sb[:, 4096:6144],
            bass.AP(tensor=pn_ext, offset=4096, ap=[[0, P], [1, 2048]]),
        )
        rows_sb = constp.tile([P, 5, RT], F32)
        nc.scalar.dma_start(
            rows_sb[:], rows_ext[:, :].rearrange("p (s r) -> p s r", s=5)
        )
        nc.scalar.dma_start(
            q_sb[:, 4096:6144],
            bass.AP(tensor=q_ext, offset=4096, ap=[[0, P], [1, 2048]]),
        )
        load_pn(6144, 2048)
        phib = constp.tile([P, RT, K], BF16)
        nc.sync.dma_start(phib[:], phib_ext[:, :].rearrange("p (r k) -> p r k", r=RT))
        load_q(6144, 2048)
        load_pn(0, 2048)
        load_pn(2048, 2048)
        load_q(0, 2048)
        load_q(2048, 2048)
        psiR = constp.tile([K * 16, 256], BF16)
        nc.sync.dma_start(psiR[:], psi_ext[:, :])
        psi56 = constp.tile([K * 8, 256], BF16)
        nc.sync.dma_start(psi56[:], psi56_ext[:, :])
        psiP = constp.tile([K, 2048], BF16)
        nc.sync.dma_start(psiP[:], psip_ext[:, :])

        t01r = rows_sb[:, 0, :]
        pr = rows_sb[:, 1, :]
        qr = rows_sb[:, 2, :]
        qneg = rows_sb[:, 3, :]
        b10 = rows_sb[:, 4, :]

        accP = sp.tile([K * 16, 4], F32)
        nc.vector.memset(accP[:], 0.0)
        ttr_scr = sp.tile([K * 16, 256], BF16)
        ttr_scr2 = sp.tile([K, 2048], BF16)

        OPMAP = {"diag": DIAG8, "diag2": DIAG2, "hard": HARD7}

        def emit_zones(r, hb, he, v, h_dve_cols=0):
            for kind, zs, ze in tiles_sched[r]["zones"]:
                zs, ze = max(zs, hb), min(ze, he)
                if zs >= ze:
                    continue
                if kind in OPMAP:
                    nc.vector._custom_dve(
                        OPMAP[kind],
                        out=v[:, zs - hb : ze - hb],
                        in0=q_sb[:, zs:ze],
                        in1=pn_sb[:, zs:ze],
                        s0=t01r[:, r : r + 1],
                        s1=pr[:, r : r + 1],
                        imm2=0.1,
                    )
                elif kind == "L":
                    nc.vector.tensor_scalar(
                        v[:, zs - hb : ze - hb],
                        q_sb[:, zs:ze],
                        qr[:, r : r + 1],
                        0.0,
                        ALU.subtract,
                        ALU.max,
                    )
                else:  # H: optionally give a leading slice to the 4x DVE path
                    zm = min(zs + h_dve_cols, ze) & ~1
                    if zm > zs:
                        nc.vector.tensor_scalar(
                            v[:, zs - hb : zm - hb],
                            pn_sb[:, zs:zm],
                            b10[:, r : r + 1],
                            0.0,
                            ALU.add,
                            ALU.max,
                        )
                    if ze > zm:
                        nc.scalar.activation(
                            v[:, zm - hb : ze - hb],
                            pn_sb[:, zm:ze],
                            RELU,
                            bias=b10[:, r : r + 1],
                            scale=1.0,
                        )

        # per-half matmul window bookkeeping (canonical schedule)
        def half_plan(hb, he, order):
            acts = {r: max(tiles_sched[r]["act"], hb) for r in order}
            members = {}
            for w0 in range(hb, he, MMF):
                we = w0 + MMF
                members[w0] = [r for r in order if acts[r] < we]
            return acts, members

        def emit_matmuls(r, hb, he, v, acts, members, X):
            c = acts[r]
            while c < he:
                we = min((c // MMF + 1) * MMF, he)
                w0 = (c // MMF) * MMF
                mm_start = members[w0][0] == r
                mm_stop = members[w0][-1] == r
                nc.tensor.matmul(
                    X[:, c - hb : we - hb],
                    phib[:, r, :],
                    v[:, c - hb : we - hb],
                    start=mm_start,
                    stop=mm_stop,
                )
                c = we

        def drain_direct(Xsb, c0, c1, col):
            # tail path: plain [K, w] multiply-reduce on SBUF copy, no DMAs
            nc.vector._custom_dve(
                dve_ops.TENSOR_TENSOR_REDUCE,
                out=ttr_scr2[:, : c1 - c0],
                in0=Xsb[:, c0:c1],
                in1=psiP[:, c0 - 2048 : c1 - 2048],
                s0=0.0,
                s1=1.0,
                accum_out=accP[:K, col : col + 1],
            )

        # ---- half 0: columns [4096, 8192), all tiles, ascending so the
        # first matmul per 512-window fully covers it (PSUM has_written) ----
        order0 = list(range(RT))
        acts0, mem0 = half_plan(HW, N, order0)
        X = pp.tile([K, HW], F32, tag="X", name="X0")
        Xsb0 = sp.tile([K, HW], BF16, tag="xsb0")

        def warm_pe(n_mm):
            # dummy matmuls ride the PE HAM activity window into the 2.4 GHz
            # state; their PSUM writes are discarded by the real start=True
            # matmul of window 0.
            for i in range(n_mm):
                nc.tensor.matmul(
                    X[:, 0:MMF],
                    phib[:, 0, :],
                    pn_sb[:, HW : HW + MMF],
                    start=(i == 0),
                    stop=(i == n_mm - 1),
                )

        warm_pe(8)
        # X0 column ranges complete progressively -> chase them with the
        # drain copy pieces so the reshape DMAs and TTR need not wait for r7.
        drain_piece = {}
        for w0 in range(HW, N, MMF):
            last = mem0[w0][-1]
            lo, hi = drain_piece.get(last, (w0 - HW, w0 - HW))
            drain_piece[last] = (min(lo, w0 - HW), w0 - HW + MMF)
        for r in order0:
            v = vp.tile([P, HW], BF16, tag="v", name=f"v0_{r}")
            emit_zones(r, HW, N, v, h_dve_cols=1536 if r <= 1 else 0)
            emit_matmuls(r, HW, N, v, acts0, mem0, X)
            if r in drain_piece:
                c0, c1 = drain_piece[r]
                nc.scalar.copy(Xsb0[:, c0:c1], X[:, c0:c1])
        xr0 = sp.tile([K * 16, 256], BF16, tag="xr0")
        for n in range(K):
            nc.sync.dma_start(
                xr0[n * 16 : (n + 1) * 16, :],
                Xsb0[n : n + 1, :].rearrange("o (k f) -> o k f", f=256),
            )

        # ---- half 1: columns [0, 4096), contributing tiles, ascending;
        # drain pieces chase the last contributor of each column range ----
        order1 = [r for r in range(RT) if tiles_sched[r]["act"] < HW]
        acts1, mem1 = half_plan(0, HW, order1)
        X1 = pp.tile([K, HW], F32, tag="X", name="X1")
        Xsb1 = sp.tile([K, HW], BF16, tag="xsb1")

        for i in range(3):
            nc.tensor.matmul(
                X1[:, 0:MMF],
                phib[:, 0, :],
                pn_sb[:, HW : HW + MMF],
                start=(i == 0),
                stop=(i == 2),
            )
        for i, r in enumerate(order1):
            v = vp.tile([P, HW], BF16, tag="v", name=f"v1_{r}")
            emit_zones(r, 0, HW, v)
            emit_matmuls(r, 0, HW, v, acts1, mem1, X1)
            if i == 3:
                nc.vector._custom_dve(
                    dve_ops.TENSOR_TENSOR_REDUCE,
                    out=ttr_scr[:],
                    in0=xr0[:],
                    in1=psiR[:],
                    s0=0.0,
                    s1=1.0,
                    accum_out=accP[:, 0:1],
                )
            if i == len(order1) - 3:
                # cols [0, 2048) complete: reshaped 56-partition drain
                nc.scalar.copy(Xsb1[:, 0:2048], X1[:, 0:2048])
                xr1 = sp.tile([K * 8, 256], BF16, tag="xr1")
                for n in range(K):
                    nc.sync.dma_start(
                        xr1[n * 8 : (n + 1) * 8, :],
                        Xsb1[n : n + 1, 0:2048].rearrange("o (k f) -> o k f", f=256),
                    )
                nc.vector._custom_dve(
                    dve_ops.TENSOR_TENSOR_REDUCE,
                    out=ttr_scr[0 : K * 8, :],
                    in0=xr1[:],
                    in1=psi56[:, :],
                    s0=0.0,
                    s1=1.0,
                    accum_out=accP[0 : K * 8, 1:2],
                )
            if i == len(order1) - 2:
                nc.scalar.copy(Xsb1[:, 2048:3584], X1[:, 2048:3584])
                drain_direct(Xsb1, 2048, 3584, 2)
        nc.scalar.copy(Xsb1[:, 3584:HW], X1[:, 3584:HW])
        drain_direct(Xsb1, 3584, HW, 3)

        nc.sync.dma_start(
            out_ext[0:448].rearrange("(p c) -> p c", c=4), accP[:]
        )

    nc.compile()
    return nc


_NC_CACHE = {}


def _exact_count(t: np.ndarray) -> int:
    n = t.shape[0]
    _, cnts = np.unique(t, return_counts=True)
    dup = int(sum(int(c) * (int(c) - 1) // 2 for c in cnts[cnts > 1]))
    return n * (n - 1) // 2 - dup


def _make_in_maps(predictions, targets, uncertainties):
    t = np.ascontiguousarray(np.asarray(targets, np.float32))
    p = np.ascontiguousarray(np.asarray(predictions, np.float32))
    u = np.ascontiguousarray(np.asarray(uncertainties, np.float32))
    order = np.argsort(t, kind="stable")
    ts, ps, us = t[order], p[order], u[order]

    q16 = (np.float32(0.1) * ts - ps).astype(np.float16)
    pn16 = (-ps).astype(np.float16)
    t01f = q16.astype(np.float32) - pn16.astype(np.float32)

    # Psi drain layouts (device half 0 = columns [4096, 8192)):
    #   psiR  [n*16+k, f] = x^n at col 4096 + k*256 + f
    #   psi56 [n*8+k, f]  = x^n at col k*256 + f          (cols [0, 2048))
    #   psiP  [n, j]      = x^n at col 2048 + j           (cols [2048, 4096))
    BF16_NP = mybir.dt.np(BF16)
    xs = us - np.float32(0.5)
    psiR = np.empty((K * 16, 256), np.float32)
    psi56 = np.empty((K * 8, 256), np.float32)
    psiP = np.empty((K, 2048), np.float32)
    hi = xs[4096:].reshape(16, 256)
    lo = xs[0:2048].reshape(8, 256)
    for n in range(K):
        psiR[n * 16 : (n + 1) * 16, :] = hi**n
        psi56[n * 8 : (n + 1) * 8, :] = lo**n
        psiP[n, :] = xs[2048:4096] ** n
    psiR_b = np.ascontiguousarray(psiR.astype(BF16_NP))
    psi56_b = np.ascontiguousarray(psi56.astype(BF16_NP))
    psiP_b = np.ascontiguousarray(psiP.astype(BF16_NP))

    in_maps = []
    for c in range(NCORES):
        pos = (1024 * np.arange(RT)[:, None] + 8 * np.arange(P)[None, :] + c).T  # [P, RT]
        t01r = t01f[pos]
        prr = -pn16.astype(np.float32)[pos]
        qrr = q16.astype(np.float32)[pos]
        rows = np.stack(
            [t01r, prr, qrr, -qrr, prr + np.float32(0.1)], axis=1
        )  # [P, 5, RT]
        xrow = xs[pos]  # [P, RT]
        pows = xrow[:, :, None] ** np.arange(K)[None, None, :]  # [P, RT, m]
        phib = np.einsum("prm,mn->prn", pows, _ACOEF)
        in_maps.append(
            {
                "qcol": q16,
                "pncol": pn16,
                "rows": np.ascontiguousarray(rows.reshape(P, 5 * RT), np.float32),
                "phib": np.ascontiguousarray(phib.reshape(P, RT * K).astype(BF16_NP)),
                "psiR": psiR_b,
                "psi56": psi56_b,
                "psiP": psiP_b,
            }
        )
    return in_maps, t, t01f


def _get_nc(t01f):
    key = hash(t01f.tobytes())
    if key not in _NC_CACHE:
        _NC_CACHE[key] = _build_nc(_make_schedule(t01f))
    return _NC_CACHE[key]


def _run_device(in_maps, t01f, trace=False, **kw):
    nc = _get_nc(t01f)
    return run_bass_kernel_spmd(
        nc, in_maps, core_ids=list(range(NCORES)), trace=trace, **kw
    )


def _host_correction(ts, ps, us, q16, pn16, t01f, sched):
    """Exact fp64 patch for the two device approximations:
    (1) pairs with identical reconstructed t01f (a_dev == 0) are dropped by
        the device indicator on BOTH orderings -> add their true value;
    (2) diag2 cells with 0 < a_dev < 0.01 use margin a instead of 0.01 ->
        add (v_true - v_dev)."""
    tf = ts.astype(np.float64)
    pf = ps.astype(np.float64)
    uf = us.astype(np.float64)
    q32 = q16.astype(np.float32)
    pn32 = pn16.astype(np.float32)
    corr = np.float64(0.0)

    # (1) equal-t01f groups
    _, inv, cnts = np.unique(t01f, return_inverse=True, return_counts=True)
    for g in np.nonzero(cnts > 1)[0]:
        idx = np.nonzero(inv == g)[0]
        ii, jj = np.triu_indices(len(idx), k=1)
        a, b = idx[ii], idx[jj]
        a32 = tf[b] - tf[a]
        live = a32 != 0.0
        if not live.any():
            continue
        a_, b_ = a[live], b[live]
        sgn = np.sign(a32[live])
        m = np.clip(0.1 * np.abs(tf[b_] - tf[a_]), 0.01, 0.1)
        hinge = np.maximum(m - sgn * (pf[b_] - pf[a_]), 0.0)
        w = 1.0 / (1.0 + uf[a_] + uf[b_])
        corr += (w * hinge).sum()

    # (2) diag2 small-margin cells
    for r, s in enumerate(sched):
        for kind, lo, hi in s["zones"]:
            if kind != "diag2":
                continue
            rows = np.arange(1024 * r, 1024 * (r + 1))
            a_dev = t01f[None, lo:hi] - t01f[rows][:, None]
            mask = (a_dev > 0) & (a_dev < 0.01)
            if not mask.any():
                continue
            ri, cj = np.nonzero(mask)
            gi, gj = rows[ri], np.arange(lo, hi)[cj]
            b_dev = (-pn32[gj]) - (-pn32[gi])
            v_dev = np.maximum(a_dev[ri, cj] - b_dev, 0.0).astype(np.float64)
            a32 = tf[gj] - tf[gi]
            v_true = np.where(
                a32 > 0,
                np.maximum(np.clip(0.1 * a32, 0.01, 0.1) - (pf[gj] - pf[gi]), 0.0),
                0.0,
            )
            w = 1.0 / (1.0 + uf[gi] + uf[gj])
            corr += (w * (v_true - v_dev)).sum()
    return corr


def kernel(predictions, targets, uncertainties):
    in_maps, t, t01f = _make_in_maps(predictions, targets, uncertainties)
    res = _run_device(in_maps, t01f)
    total = np.float64(0.0)
    for r in res.results:
        total += np.asarray(r["out"], np.float64).sum()
    order = np.argsort(t, kind="stable")
    ts = t[order]
    ps = np.ascontiguousarray(np.asarray(predictions, np.float32))[order]
    us = np.ascontiguousarray(np.asarray(uncertainties, np.float32))[order]
    q16 = (np.float32(0.1) * ts - ps).astype(np.float16)
    pn16 = (-ps).astype(np.float16)
    total += _host_correction(ts, ps, us, q16, pn16, t01f, _make_schedule(t01f))
    count = _exact_count(t)
    return np.asarray(total / max(count, 1), dtype=np.float32)


# revision 27
# speedup vs baseline: 1.1715x; 1.1715x over previous
"""AdaptiveRankingLoss distributed Bass kernel for 8 TRN2 NeuronCores (v2).

Math
----
reference loss = sum_{i<j, t_i != t_j} w_ij * relu(m_ij - sgn(t_i-t_j)(p_i-p_j))
                 / count,   m = 0.1*clip(|t_i-t_j|, 0.1, 1.0), w = 1/(1+u_i+u_j).

Sorting by t and splitting by the sign of as = 0.1*(t_j - t_i) gives the
full-matrix form  sum_{as>0} w * relu(clip(as, .01, .1) - (p_j - p_i)).

Columns are streamed as two fp16 broadcast tensors q = 0.1 t - p and
pn = -p; rows enter as per-partition scalars.  Each 128-row tile (strided
over 1024 consecutive sorted positions, identical span on every core so
one SPMD program serves all 8 cores) splits its column range into zones
classified host-side from the device-exact a-values t01f = q - pn:

  diag  [ind uncertain]          custom DVE op (indicator, max-margin)
  hard  [margin clip uncertain]  custom DVE op (full clip, no indicator)
  L     [m = as exactly]         v = relu(q_j - q_i)        -> stock
                                 tensor_scalar (sub, max0) at 4x DVE mode
  H     [m = 0.1]                v = relu(pn_j + p_i + 0.1) -> ScalarE
                                 activation Relu with per-partition bias

The weight w is applied through the degree-6 bilinear split
w ~ sum_n Phi_n(x_i) Psi_n(x_j), x = u - .5:  TensorEngine contracts
Phi against the v tiles into PSUM X[n, j] (PSUM halves, start/stop
bookkeeping per 512-column bank window), X is drained via a partition
reshape and a fused multiply-reduce against Psi.  Phi and Psi are
precomputed on the host and DMAed in directly.
"""

import numpy as np

import concourse.bass as bass
import concourse.bacc as bacc
import concourse.mybir as mybir
import concourse.tile as tile
from concourse.bass_utils import run_bass_kernel_spmd
from concourse import dve_ops
from concourse.dve_spec import (
    Spec,
    Src0,
    Src1,
    C0,
    C1,
    C2,
    Zero,
    relu,
    maxx,
    minn,
    lower,
    _has_src1,
)
from concourse.dve_uop import DveOpSpec

F32 = mybir.dt.float32
F16 = mybir.dt.float16
BF16 = mybir.dt.bfloat16
ALU = mybir.AluOpType
RELU = mybir.ActivationFunctionType.Relu

N = 8192
NCORES = 8
P = 128
R = N // NCORES      # rows per core (1024)
RT = R // P          # row tiles per core (8)
HW = N // 2          # column half width (4096)
DEG = 6
K = DEG + 1
MMF = 512            # matmul free-dim / PSUM bank window
EPS = 1e-6


# --------------------------------------------------------------------------
# custom DVE ops (Src0 = q col, Src1 = pn col, C0 = t01 row, C1 = p row)
#   a  = (Src0 - Src1) - C0          exact scaled target difference
#   b  = p_j - p_i = -(Src1 + C1)    so  m - b = m + Src1 + C1
# --------------------------------------------------------------------------
def _register_op(name, spec):
    for op in dve_ops.OPS:
        if op.name == name:
            return op
    row = dve_ops._CUSTOM_DVE_ROW_BASE + len(dve_ops.OPS)
    assert row < 0x20, "custom-DVE row overflow"
    dve_ops._SUB_OPCODE_FOR_NAME[name] = row
    shas = {}
    for ver in ("v3", "v4"):
        try:
            uops = lower(spec, ver=ver)
            shas[ver] = DveOpSpec(
                name=name, opcode=row, uops=uops, rd1_en=_has_src1(spec)
            ).sha(ver)
        except Exception:
            pass
    op = dve_ops.DveOp(name, spec, subdim=False, uops_sha=shas)
    dve_ops.OPS.append(op)
    dve_ops.CUSTOM_DVE_SPECS[name] = spec
    return op


def _hard_ref(in0, in1, s0, s1, imm2):
    a = (in0 - in1) - s0
    m = np.clip(a, np.float32(imm2) * np.float32(imm2), imm2)
    return np.maximum(m + in1 + s1, 0.0)


def _diag_ref(in0, in1, s0, s1, imm2):
    a = (in0 - in1) - s0
    m = np.maximum(a, np.float32(imm2) * np.float32(imm2))
    return (a > 0).astype(np.float32) * np.maximum(m + in1 + s1, 0.0)


def _diag2_ref(in0, in1, s0, s1, imm2):
    a = (in0 - in1) - s0
    m = np.minimum(a, np.float32(imm2))
    return (a > 0).astype(np.float32) * np.maximum(m + in1 + s1, 0.0)


_A = (Src0 - Src1) - C0
HARD7 = _register_op(
    "ARL_HARD_V2",
    Spec(body=relu((minn(maxx(_A, C2 * C2), C2) + Src1) + C1), reference=_hard_ref),
)
DIAG8 = _register_op(
    "ARL_DIAG_V2",
    Spec(
        body=(_A > Zero) * relu((maxx(_A, C2 * C2) + Src1) + C1), reference=_diag_ref
    ),
)
DIAG2 = _register_op(
    "ARL_DIAG2_V2",
    Spec(body=(_A > Zero) * relu((minn(_A, C2) + Src1) + C1), reference=_diag2_ref),
)


# --------------------------------------------------------------------------
# degree-6 bilinear split of w = 1/(2 + x_i + x_j), x = u - .5
# --------------------------------------------------------------------------
def _acoef_matrix() -> np.ndarray:
    from numpy.polynomial import chebyshev as _C
    from math import comb

    nodes = np.cos((2 * np.arange(DEG + 1) + 1) / (2 * (DEG + 1)) * np.pi)
    ch = _C.chebfit(nodes, 1.0 / (2.0 + nodes), DEG)
    c = _C.cheb2poly(ch)
    A = np.zeros((K, K), np.float64)
    for mm in range(K):
        for nn in range(K):
            if mm + nn <= DEG:
                A[mm, nn] = c[mm + nn] * comb(mm + nn, mm)
    return A.astype(np.float32)


_ACOEF = _acoef_matrix()


# --------------------------------------------------------------------------
# host-side zone classification (canonical across cores)
# --------------------------------------------------------------------------
def _first_above(arr, val):
    return int(np.searchsorted(arr, val, side="right"))


def _make_schedule(t01f):
    """Per row tile: ordered [(kind, lo, hi)] zones covering [act, N)."""
    cmax = np.maximum.accumulate(t01f)
    csuf = np.minimum.accumulate(t01f[::-1])[::-1]
    tiles = []
    for r in range(RT):
        span = t01f[1024 * r : 1024 * (r + 1)]
        tmin, tmax = float(span.min()), float(span.max())
        act = 0 if r == 0 else _first_above(cmax, tmin - EPS)
        d1 = _first_above(csuf, tmax + EPS)        # a > 0 certain from here
        ds = _first_above(cmax, tmin + 0.1 - EPS)  # diag max-form valid before
        l0 = _first_above(csuf, tmax + 0.01 + EPS)
        l1 = ds
        h0 = _first_above(csuf, tmax + 0.1 + EPS)
        act &= ~1
        d1 = min((d1 + 1) & ~1, N)
        ds_e = min(max((ds + 1) & ~1, act), d1)
        l0 = min(max((l0 + 1) & ~1, d1), N)
        l1 = max(l1 & ~1, l0)
        h0 = min(max((h0 + 1) & ~1, l1), N)
        zones = []
        if ds_e > act:
            zones.append(("diag", act, ds_e))
        if d1 > ds_e:
            zones.append(("diag2", ds_e, d1))
        if l0 > d1:
            zones.append(("hard", d1, l0))
        if l1 > l0:
            zones.append(("L", l0, l1))
        if h0 > l1:
            zones.append(("hard", l1, h0))
        if N > h0:
            zones.append(("H", h0, N))
        tiles.append({"act": act, "zones": zones})
    return tiles


# --------------------------------------------------------------------------
# device graph
# --------------------------------------------------------------------------
def _build_nc(tiles_sched):
    from contextlib import ExitStack

    nc = bacc.Bacc(None, target_bir_lowering=False, debug=False)

    q_ext = nc.declare_dram_parameter("qcol", [N], F16, isOutput=False)
    pn_ext = nc.declare_dram_parameter("pncol", [N], F16, isOutput=False)
    rows_ext = nc.declare_dram_parameter("rows", [P, 5 * RT], F32, isOutput=False)
    phib_ext = nc.declare_dram_parameter("phib", [P, RT * K], BF16, isOutput=False)
    psi_ext = nc.declare_dram_parameter("psiR", [K * 16, 256], BF16, isOutput=False)
    psi56_ext = nc.declare_dram_parameter("psi56", [K * 8, 256], BF16, isOutput=False)
    psip_ext = nc.declare_dram_parameter("psiP", [K, 2048], BF16, isOutput=False)
    out_ext = nc.declare_dram_parameter("out", [448], F32, isOutput=True)

    with tile.TileContext(nc) as tc, ExitStack() as ctx:
        constp = ctx.enter_context(tc.tile_pool(name="const", bufs=1))
        colp = ctx.enter_context(tc.tile_pool(name="cols", bufs=1))
        vp = ctx.enter_context(tc.tile_pool(name="v", bufs=6))
        pp = ctx.enter_context(tc.tile_pool(name="psum", bufs=1, space="PSUM"))
        sp = ctx.enter_context(tc.tile_pool(name="small", bufs=1))

        # ---- column tensors (fp16, broadcast to all partitions) ----
        q_sb = colp.tile([P, N], F16)
        pn_sb = colp.tile([P, N], F16)

        def load_q(lo, w):
            nc.sync.dma_start(
                q_sb[:, lo : lo + w],
                bass.AP(tensor=q_ext, offset=lo, ap=[[0, P], [1, w]]),
            )

        def load_pn(lo, w):
            nc.sync.dma_start(
                pn_sb[:, lo : lo + w],
                bass.AP(tensor=pn_ext, offset=lo, ap=[[0, P], [1, w]]),
            )

        # first wave via the ScalarE HWDGE queue — it is idle right after the
        # entry barrier, ~3 µs before the Sync queue issues its first DMA
        nc.scalar.dma_start(
            pn_sb[:, 4096:6144],
            bass.AP(tensor=pn_ext, offset=4096, ap=[[0, P], [1, 2048]]),
        )
        rows_sb = constp.tile([P, 5, RT], F32)
        nc.scalar.dma_start(
            rows_sb[:], rows_ext[:, :].rearrange("p (s r) -> p s r", s=5)
        )
        nc.scalar.dma_start(
            q_sb[:, 4096:6144],
            bass.AP(tensor=q_ext, offset=4096, ap=[[0, P], [1, 2048]]),
        )
        load_pn(6144, 2048)
        phib = constp.tile([P, RT, K], BF16)
        nc.sync.dma_start(phib[:], phib_ext[:, :].rearrange("p (r k) -> p r k", r=RT))
        load_q(6144, 2048)
        load_pn(0, 2048)
        load_pn(2048, 2048)
        load_q(0, 2048)
        load_q(2048, 2048)
        psiR = constp.tile([K * 16, 256], BF16)
        nc.sync.dma_start(psiR[:], psi_ext[:, :])
        psi56 = constp.tile([K * 8, 256], BF16)
        nc.sync.dma_start(psi56[:], psi56_ext[:, :])
        psiP = constp.tile([K, 2048], BF16)
        nc.sync.dma_start(psiP[:], psip_ext[:, :])

        t01r = rows_sb[:, 0, :]
        pr = rows_sb[:, 1, :]
        qr = rows_sb[:, 2, :]
        qneg = rows_sb[:, 3, :]
        b10 = rows_sb[:, 4, :]

        accP = sp.tile([K * 16, 4], F32)
        nc.vector.memset(accP[:], 0.0)
        ttr_scr = sp.tile([K * 16, 256], BF16)
        ttr_scr2 = sp.tile([K, 2048], BF16)

        OPMAP = {"diag": DIAG8, "diag2": DIAG2, "hard": HARD7}

        def emit_zones(r, hb, he, v, h_dve_cols=0):
            for kind, zs, ze in tiles_sched[r]["zones"]:
                zs, ze = max(zs, hb), min(ze, he)
                if zs >= ze:
                    continue
                if kind in OPMAP:
                    nc.vector._custom_dve(
                        OPMAP[kind],
                        out=v[:, zs - hb : ze - hb],
                        in0=q_sb[:, zs:ze],
                        in1=pn_sb[:, zs:ze],
                        s0=t01r[:, r : r + 1],
                        s1=pr[:, r : r + 1],
                        imm2=0.1,
                    )
                elif kind == "L":
                    nc.vector.tensor_scalar(
                        v[:, zs - hb : ze - hb],
                        q_sb[:, zs:ze],
                        qr[:, r : r + 1],
                        0.0,
                        ALU.subtract,
                        ALU.max,
                    )
                else:  # H: optionally give a leading slice to the 4x DVE path
                    zm = min(zs + h_dve_cols, ze) & ~1
                    if zm > zs:
                        nc.vector.tensor_scalar(
                            v[:, zs - hb : zm - hb],
                            pn_sb[:, zs:zm],
                            b10[:, r : r + 1],
                            0.0,
                            ALU.add,
                            ALU.max,
                        )
                    if ze > zm:
                        nc.scalar.activation(
                            v[:, zm - hb : ze - hb],
                            pn_sb[:, zm:ze],
                            RELU,
                            bias=b10[:, r : r + 1],
                            scale=1.0,
                        )

        # per-half matmul window bookkeeping (canonical schedule)
        def half_plan(hb, he, order):
            acts = {r: max(tiles_sched[r]["act"], hb) for r in order}
            members = {}
            for w0 in range(hb, he, MMF):
                we = w0 + MMF
                members[w0] = [r for r in order if acts[r] < we]
            return acts, members

        def emit_matmuls(r, hb, he, v, acts, members, X):
            c = acts[r]
            while c < he:
                we = min((c // MMF + 1) * MMF, he)
                w0 = (c // MMF) * MMF
                mm_start = members[w0][0] == r
                mm_stop = members[w0][-1] == r
                nc.tensor.matmul(
                    X[:, c - hb : we - hb],
                    phib[:, r, :],
                    v[:, c - hb : we - hb],
                    start=mm_start,
                    stop=mm_stop,
                )
                c = we

        def drain_direct(Xsb, c0, c1, col):
            # tail path: plain [K, w] multiply-reduce on SBUF copy, no DMAs
            nc.vector._custom_dve(
                dve_ops.TENSOR_TENSOR_REDUCE,
                out=ttr_scr2[:, : c1 - c0],
                in0=Xsb[:, c0:c1],
                in1=psiP[:, c0 - 2048 : c1 - 2048],
                s0=0.0,
                s1=1.0,
                accum_out=accP[:K, col : col + 1],
            )

        # ---- half 0: columns [4096, 8192), all tiles, ascending so the
        # first matmul per 512-window fully covers it (PSUM has_written) ----
        order0 = list(range(RT))
        acts0, mem0 = half_plan(HW, N, order0)
        X = pp.tile([K, HW], F32, tag="X", name="X0")
        Xsb0 = sp.tile([K, HW], BF16, tag="xsb0")

        def warm_pe(n_mm):
            # dummy matmuls ride the PE HAM activity window into the 2.4 GHz
            # state; their PSUM writes are discarded by the real start=True
            # matmul of window 0.
            for i in range(n_mm):
                nc.tensor.matmul(
                    X[:, 0:MMF],
                    phib[:, 0, :],
                    pn_sb[:, HW : HW + MMF],
                    start=(i == 0),
                    stop=(i == n_mm - 1),
                )

        warm_pe(8)
        for r in order0:
            v = vp.tile([P, HW], BF16, tag="v", name=f"v0_{r}")
            emit_zones(r, HW, N, v, h_dve_cols=1536 if r <= 1 else 0)
            emit_matmuls(r, HW, N, v, acts0, mem0, X)
        # piecewise ACT copy after the loop: frees PSUM for half 1 early
        # without head-of-line blocking the ACT queue mid-half
        for c0 in range(0, HW, 1024):
            nc.scalar.copy(Xsb0[:, c0 : c0 + 1024], X[:, c0 : c0 + 1024])
        xr0 = sp.tile([K * 16, 256], BF16, tag="xr0")
        for n in range(K):
            nc.sync.dma_start(
                xr0[n * 16 : (n + 1) * 16, :],
                Xsb0[n : n + 1, :].rearrange("o (k f) -> o k f", f=256),
            )

        # ---- half 1: columns [0, 4096), contributing tiles, ascending;
        # drain pieces chase the last contributor of each column range ----
        order1 = [r for r in range(RT) if tiles_sched[r]["act"] < HW]
        acts1, mem1 = half_plan(0, HW, order1)
        X1 = pp.tile([K, HW], F32, tag="X", name="X1")
        Xsb1 = sp.tile([K, HW], BF16, tag="xsb1")

        for i in range(3):
            nc.tensor.matmul(
                X1[:, 0:MMF],
                phib[:, 0, :],
                pn_sb[:, HW : HW + MMF],
                start=(i == 0),
                stop=(i == 2),
            )
        for i, r in enumerate(order1):
            v = vp.tile([P, HW], BF16, tag="v", name=f"v1_{r}")
            emit_zones(r, 0, HW, v)
            emit_matmuls(r, 0, HW, v, acts1, mem1, X1)
            if i == 3:
                nc.vector._custom_dve(
                    dve_ops.TENSOR_TENSOR_REDUCE,
                    out=ttr_scr[:],
                    in0=xr0[:],
                    in1=psiR[:],
                    s0=0.0,
                    s1=1.0,
                    accum_out=accP[:, 0:1],
                )
            if i == len(order1) - 3:
                # cols [0, 2048) complete: reshaped 56-partition drain
                nc.scalar.copy(Xsb1[:, 0:2048], X1[:, 0:2048])
                xr1 = sp.tile([K * 8, 256], BF16, tag="xr1")
                for n in range(K):
                    nc.sync.dma_start(
                        xr1[n * 8 : (n + 1) * 8, :],
                        Xsb1[n : n + 1, 0:2048].rearrange("o (k f) -> o k f", f=256),
                    )
                nc.vector._custom_dve(
                    dve_ops.TENSOR_TENSOR_REDUCE,
                    out=ttr_scr[0 : K * 8, :],
                    in0=xr1[:],
                    in1=psi56[:, :],
                    s0=0.0,
                    s1=1.0,
                    accum_out=accP[0 : K * 8, 1:2],
                )
            if i == len(order1) - 2:
                nc.scalar.copy(Xsb1[:, 2048:3584], X1[:, 2048:3584])
                drain_direct(Xsb1, 2048, 3584, 2)
        nc.scalar.copy(Xsb1[:, 3584:HW], X1[:, 3584:HW])
        drain_direct(Xsb1, 3584, HW, 3)

        nc.sync.dma_start(
            out_ext[0:448].rearrange("(p c) -> p c", c=4), accP[:]
        )

    nc.compile()
    return nc


_NC_CACHE = {}


def _exact_count(t: np.ndarray) -> int:
    n = t.shape[0]
    _, cnts = np.unique(t, return_counts=True)
    dup = int(sum(int(c) * (int(c) - 1) // 2 for c in cnts[cnts > 1]))
    return n * (n - 1) // 2 - dup


def _make_in_maps(predictions, targets, uncertainties):
    t = np.ascontiguousarray(np.asarray(targets, np.float32))
    p = np.ascontiguousarray(np.asarray(predictions, np.float32))
    u = np.ascontiguousarray(np.asarray(uncertainties, np.float32))
    order = np.argsort(t, kind="stable")
    ts, ps, us = t[order], p[order], u[order]

    q16 = (np.float32(0.1) * ts - ps).astype(np.float16)
    pn16 = (-ps).astype(np.float16)
    t01f = q16.astype(np.float32) - pn16.astype(np.float32)

    # Psi drain layouts (device half 0 = columns [4096, 8192)):
    #   psiR  [n*16+k, f] = x^n at col 4096 + k*256 + f
    #   psi56 [n*8+k, f]  = x^n at col k*256 + f          (cols [0, 2048))
    #   psiP  [n, j]      = x^n at col 2048 + j           (cols [2048, 4096))
    BF16_NP = mybir.dt.np(BF16)
    xs = us - np.float32(0.5)
    psiR = np.empty((K * 16, 256), np.float32)
    psi56 = np.empty((K * 8, 256), np.float32)
    psiP = np.empty((K, 2048), np.float32)
    hi = xs[4096:].reshape(16, 256)
    lo = xs[0:2048].reshape(8, 256)
    for n in range(K):
        psiR[n * 16 : (n + 1) * 16, :] = hi**n
        psi56[n * 8 : (n + 1) * 8, :] = lo**n
        psiP[n, :] = xs[2048:4096] ** n
    psiR_b = np.ascontiguousarray(psiR.astype(BF16_NP))
    psi56_b = np.ascontiguousarray(psi56.astype(BF16_NP))
    psiP_b = np.ascontiguousarray(psiP.astype(BF16_NP))

    in_maps = []
    for c in range(NCORES):
        pos = (1024 * np.arange(RT)[:, None] + 8 * np.arange(P)[None, :] + c).T  # [P, RT]
        t01r = t01f[pos]
        prr = -pn16.astype(np.float32)[pos]
        qrr = q16.astype(np.float32)[pos]
        rows = np.stack(
            [t01r, prr, qrr, -qrr, prr + np.float32(0.1)], axis=1
        )  # [P, 5, RT]
        xrow = xs[pos]  # [P, RT]
        pows = xrow[:, :, None] ** np.arange(K)[None, None, :]  # [P, RT, m]
        phib = np.einsum("prm,mn->prn", pows, _ACOEF)
        in_maps.append(
            {
                "qcol": q16,
                "pncol": pn16,
                "rows": np.ascontiguousarray(rows.reshape(P, 5 * RT), np.float32),
                "phib": np.ascontiguousarray(phib.reshape(P, RT * K).astype(BF16_NP)),
                "psiR": psiR_b,
                "psi56": psi56_b,
                "psiP": psiP_b,
            }
        )
    return in_maps, t, t01f


def _get_nc(t01f):
    key = hash(t01f.tobytes())
    if key not in _NC_CACHE:
        _NC_CACHE[key] = _build_nc(_make_schedule(t01f))
    return _NC_CACHE[key]


def _run_device(in_maps, t01f, trace=False, **kw):
    nc = _get_nc(t01f)
    return run_bass_kernel_spmd(
        nc, in_maps, core_ids=list(range(NCORES)), trace=trace, **kw
    )


def _host_correction(ts, ps, us, q16, pn16, t01f, sched):
    """Exact fp64 patch for the two device approximations:
    (1) pairs with identical reconstructed t01f (a_dev == 0) are dropped by
        the device indicator on BOTH orderings -> add their true value;
    (2) diag2 cells with 0 < a_dev < 0.01 use margin a instead of 0.01 ->
        add (v_true - v_dev)."""
    tf = ts.astype(np.float64)
    pf = ps.astype(np.float64)
    uf = us.astype(np.float64)
    q32 = q16.astype(np.float32)
    pn32 = pn16.astype(np.float32)
    corr = np.float64(0.0)

    # (1) equal-t01f groups
    _, inv, cnts = np.unique(t01f, return_inverse=True, return_counts=True)
    for g in np.nonzero(cnts > 1)[0]:
        idx = np.nonzero(inv == g)[0]
        ii, jj = np.triu_indices(len(idx), k=1)
        a, b = idx[ii], idx[jj]
        a32 = tf[b] - tf[a]
        live = a32 != 0.0
        if not live.any():
            continue
        a_, b_ = a[live], b[live]
        sgn = np.sign(a32[live])
        m = np.clip(0.1 * np.abs(tf[b_] - tf[a_]), 0.01, 0.1)
        hinge = np.maximum(m - sgn * (pf[b_] - pf[a_]), 0.0)
        w = 1.0 / (1.0 + uf[a_] + uf[b_])
        corr += (w * hinge).sum()

    # (2) diag2 small-margin cells
    for r, s in enumerate(sched):
        for kind, lo, hi in s["zones"]:
            if kind != "diag2":
                continue
            rows = np.arange(1024 * r, 1024 * (r + 1))
            a_dev = t01f[None, lo:hi] - t01f[rows][:, None]
            mask = (a_dev > 0) & (a_dev < 0.01)
            if not mask.any():
                continue
            ri, cj = np.nonzero(mask)
            gi, gj = rows[ri], np.arange(lo, hi)[cj]
            b_dev = (-pn32[gj]) - (-pn32[gi])
            v_dev = np.maximum(a_dev[ri, cj] - b_dev, 0.0).astype(np.float64)
            a32 = tf[gj] - tf[gi]
            v_true = np.where(
                a32 > 0,
                np.maximum(np.clip(0.1 * a32, 0.01, 0.1) - (pf[gj] - pf[gi]), 0.0),
                0.0,
            )
            w = 1.0 / (1.0 + uf[gi] + uf[gj])
            corr += (w * (v_true - v_dev)).sum()
    return corr


def kernel(predictions, targets, uncertainties):
    in_maps, t, t01f = _make_in_maps(predictions, targets, uncertainties)
    res = _run_device(in_maps, t01f)
    total = np.float64(0.0)
    for r in res.results:
        total += np.asarray(r["out"], np.float64).sum()
    order = np.argsort(t, kind="stable")
    ts = t[order]
    ps = np.ascontiguousarray(np.asarray(predictions, np.float32))[order]
    us = np.ascontiguousarray(np.asarray(uncertainties, np.float32))[order]
    q16 = (np.float32(0.1) * ts - ps).astype(np.float16)
    pn16 = (-ps).astype(np.float16)
    total += _host_correction(ts, ps, us, q16, pn16, t01f, _make_schedule(t01f))
    count = _exact_count(t)
    return np.asarray(total / max(count, 1), dtype=np.float32)
